# revision 23
# baseline (speedup 1.0000x reference)
"""Trainium2 Bass kernel for nn_BondOutputModule (gnn_message_passing).

Reference computation:
    hv = h @ W_out                       (projection pulled before segment sum)
    out[t,b] = sum_{e in type t, graph b} hv[src_e]
    graph_v[b,t]; mask; softmax over t

Device strategy (8 cores, SPMD), v8:
  - Edges of each bin (t,seg) are dealt round-robin across the 8 cores, so
    per-core bin counts are cnt_chip/8 +-1.5; K=9 pair-columns per bin plus a
    tiny extension region absorbs the tail, giving a hole-free linear layout.
  - h is supplied host-transposed in two fp16 halves [96, 12544]; PE matmuls
    with replicated-weight lhsT produce hv in PSUM; row 0 goes to DRAM.
  - AllGather hv -> [100352]; each 16-block replicated 4x -> [6272, 64] f32
    table (1.6MB - small table keeps HBM row locality for random 256B reads).
  - Slot grid [128, F']: pair (p//2, f') holds 2 same-bin edges.  Per gather
    block: dma_gather on queues 1-3 only (queue 0 measured 10x slower), DVE
    16-wide one-hot select -> fp16 vals, ONE matmul with fixed mask2 lhsT
    [128,64] sums pairs into psum[64, CB] (13 matmuls total).
  - psum blocks -> DRAM [64*F'] -> linear reshape -> sbuf [128, 72*K+2],
    one DVE reduce over K + 2 masked adds for the extension -> per-core bins
    [128,72] (bin = p*72+e).  AllReduce [9216] -> [36,256] -> PE transpose,
    mask + softmax.
"""
import sys

if '/opt/trn_rl_repo' not in sys.path:
    sys.path.insert(0, '/opt/trn_rl_repo')

import numpy as np

TRACE = [False]          # test harness can set kernel.TRACE[0] = True
LAST_EXEC_NS = [None]    # filled when TRACE is on

N = 100000
D = 192
T = 36
E = 30000
B = 256
NCORES = 8
NSH = N // NCORES          # 12500 nodes per core
NSHP = 12544               # padded (multiple of 448 and 128)
NHV = NSHP * NCORES        # 100352 positions in the AllGather'd hv
NROW = NHV // 16           # 6272 table rows (16 f32 replicated 4x = 256B)
NBINS = T * B              # 9216
BPP = NBINS // 128         # 72 bins per partition
BPO = NBINS // 64          # 144 bins per pair-row
EXTP = 4                   # extension slots per partition
EXTC = 2 * EXTP            # extension columns in the pair grid
NBLK = 13                  # gather blocks
HCHUNK = 448               # phase-1 nodes per matmul
HLOAD = 4                  # h chunks per DMA
PAD_OFF = 99.0


def _patch_tile_drain():
    """This walrus build accepts at most one sync-wait per CTRL/DMA
    instruction; Tile's tail drain can carry one wait per DMA lane."""
    import concourse.tile as tile
    from concourse.vector_clock import ScopedClock
    from concourse import mybir

    if getattr(tile.TileContext, '_bondout_patched', False):
        return

    def _drain_and_barrier(self, tick_clock, wait_clock):
        nc = self.nc
        carriers = [nc.sync.nop(nofuse=True, hint=f"dw{i}") for i in range(24)]
        drain_inst = nc.sync.drain()
        wait_clock.add_sem_waits(
            drain_inst.ins, ScopedClock({None: tick_clock.global_clock})
        )
        waits = list(drain_inst.ins.sync_info.on_wait)
        if len(waits) > 1:
            drain_inst.ins.sync_info.on_wait = waits[-1:]
            for c, w in zip(carriers, waits[:-1]):
                if c.ins.sync_info is None:
                    c.ins.sync_info = mybir.SyncInfo(on_wait=[w], on_update=[])
                else:
                    c.ins.sync_info.on_wait = [w]
        nc.all_engine_barrier()
        assert self.sems is not None
        popped = nc._tile_sem_poison_stack.pop()
        assert popped is self._sem_poison
        nc.clear_and_free_semaphores(list(self.sems.allocated().values()))
        nc.all_engine_barrier()

    tile.TileContext._drain_and_barrier = _drain_and_barrier
    tile.TileContext._bondout_patched = True


def _split_multi_waits(nc):
    from concourse import mybir
    for f in nc.m.functions:
        for blk in f.blocks:
            new = []
            changed = False
            for inst in blk.instructions:
                si = inst.sync_info
                if si is not None and si.on_wait and len(si.on_wait) > 1:
                    waits = list(si.on_wait)
                    for j, w in enumerate(waits[:-1]):
                        nop = mybir.InstNoOp(
                            name=f"{inst.name}-ws{j}",
                            engine=inst.engine,
                            bass_nofuse=True,
                            sync_info=mybir.SyncInfo(on_wait=[w], on_update=[]),
                        )
                        new.append(nop)
                    si.on_wait = waits[-1:]
                    changed = True
                new.append(inst)
            if changed:
                blk.instructions = new


def _prepare_edges(edge_src, edge_seg):
    """Deal edges round-robin per bin across cores; build per-core slot
    arrays for the K-regular pair grid plus a small extension region.

    Grid: bin b owns pairs (po=b//144, f' in [(b%144)K, +K)) holding up to
    2K edges.  Overflow (cnt>2K) spills to the extension: EXTC columns at
    f' >= 144K, whose pair (po, 144K + 2j + (p1&1)) post-repack lands at
    sb_ext[p, j] for partition p = 2po + bit; each partition owns EXTP
    extension sums, each tied to one of its own 72 bins via a host mask.

    Returns (K, per_core, ext_masks) where per_core[k] = dict(row, off) and
    ext_masks[j] = [128, 72] f32 shared mask... (per-core masks differ, so
    ext masks are per core: per_core[k]['extm'] = [EXTP, 128, 72]).
    """
    src = edge_src.astype(np.int64)
    seg = edge_seg.astype(np.int64)

    k_n = src // NSH
    q = k_n * NSHP + (src - k_n * NSH)               # [T, E] table positions

    bounds = np.empty((T, B + 1), np.int64)
    for t in range(T):
        bounds[t] = np.searchsorted(seg[t], np.arange(B + 1))
    counts = bounds[:, 1:] - bounds[:, :-1]                    # [T, B]
    e_idx = np.arange(E)[None, :]
    rank = e_idx - bounds[:, :-1][np.arange(T)[:, None], seg]
    binid = np.arange(T, dtype=np.int64)[:, None] * B + seg     # [T, E]
    core = (rank + binid) % NCORES        # rotate so ceil spreads over cores
    j = rank // NCORES                                          # per-core rank

    cnt_core_max = int(-(-counts.max() // NCORES))
    rows = (q >> 4).astype(np.int16)
    offs = (q & 15).astype(np.float32)
    po = binid // BPO

    for K in range(max(9, -(-(cnt_core_max - 2) // 2)), 16):
        Fp = -(-(BPO * K + EXTC) // NBLK) * NBLK
        in_grid = j < 2 * K
        p_g = 2 * po + (j % 2)
        fp_g = (binid % BPO) * K + np.where(in_grid, j, 0) // 2

        per_core = []
        ok = True
        for k in range(NCORES):
            m = (core == k) & in_grid
            row_a = np.zeros((128, Fp), np.int16)
            off_a = np.full((128, Fp), PAD_OFF, np.float32)
            row_a[p_g[m], fp_g[m]] = rows[m]
            off_a[p_g[m], fp_g[m]] = offs[m]

            # extension: overflow edges (<=2 per bin), one pair-column per
            # ext bin; column x=EXTP*(p_b&1)+sj of the ext region lands at
            # sb_ext[p_b, sj] after the [64,EXTC]->[128,EXTP] repack.
            extm = np.zeros((EXTP, 128, 72), np.float32)
            mo = (core == k) & ~in_grid
            if mo.any():
                ob = np.unique(binid[mo])
                slot_used = np.zeros(128, np.int64)
                for b in ob:
                    p_b = int(b // BPP)
                    sj = int(slot_used[p_b])
                    if sj >= EXTP:
                        ok = False
                        break
                    slot_used[p_b] = sj + 1
                    extm[sj, p_b, b % BPP] = 1.0
                    eb = mo & (binid == b)
                    ers = rows[eb]
                    eos = offs[eb]
                    if len(ers) > 2:
                        ok = False
                        break
                    fpx = BPO * K + EXTP * (p_b & 1) + sj
                    for i, (r_, o_) in enumerate(zip(ers, eos)):
                        row_a[2 * (p_b // 2) + i, fpx] = r_
                        off_a[2 * (p_b // 2) + i, fpx] = o_
            if not ok:
                break
            per_core.append({"row": row_a.T.copy(), "off": off_a,
                             "extm": extm})
        if ok:
            return K, per_core
    raise AssertionError("no feasible K")


def _wrap_idx(idx16, Fp):
    """dma_gather index layout: per block of PERB idxs, idx i lives at
    partition 16*core + i%16, column i//16, replicated for all 8 Q7 cores.
    idx16 is [F', 128] in (c, p) slot order (i = c*128 + p)."""
    CB = Fp // NBLK
    PERB = CB * 128
    out = np.zeros((128, Fp * 128 // 16), np.int16)
    flat = idx16.reshape(-1)
    for b in range(NBLK):
        blk = flat[b * PERB:(b + 1) * PERB]
        w = blk.reshape(PERB // 16, 16).T
        cols = slice(b * (PERB // 16), (b + 1) * (PERB // 16))
        for core in range(8):
            out[core * 16:(core + 1) * 16, cols] = w
    return out


def _build_program(K):
    import concourse.bass as bass
    from concourse import bacc, mybir
    import concourse.tile as tile

    _patch_tile_drain()
    FP = mybir.dt.float32
    F16 = mybir.dt.float16
    I16 = mybir.dt.int16
    Fp = -(-(BPO * K + EXTC) // NBLK) * NBLK
    FpR = BPO * K              # regular columns
    CB = Fp // NBLK
    PERB = CB * 128
    assert NSHP % (HCHUNK * HLOAD) == 0
    NH = NSHP // HCHUNK
    NL = NSHP // (HCHUNK * HLOAD)

    nc = bacc.Bacc(num_swdge_queues=4)
    ht0_in = nc.dram_tensor("ht0", [96, NSHP], F16, kind="ExternalInput")
    ht1_in = nc.dram_tensor("ht1", [96, NSHP], F16, kind="ExternalInput")
    w0_in = nc.dram_tensor("w0", [96, 128], F16, kind="ExternalInput")
    w1_in = nc.dram_tensor("w1", [96, 128], F16, kind="ExternalInput")
    idx_in = nc.dram_tensor("idx16", [128, Fp * 128 // 16], I16,
                            kind="ExternalInput")
    off_in = nc.dram_tensor("off", [128, Fp], FP, kind="ExternalInput")
    iota_in = nc.dram_tensor("iota16", [128, 16], FP, kind="ExternalInput")
    m2_in = nc.dram_tensor("mask2", [128, 64], F16, kind="ExternalInput")
    extm_in = nc.dram_tensor("extm", [128, EXTP * 72], FP,
                             kind="ExternalInput")
    eye_in = nc.dram_tensor("eye36", [36, 36], FP, kind="ExternalInput")
    m0_in = nc.dram_tensor("mask_keep", [128, 72], FP, kind="ExternalInput")
    mn_in = nc.dram_tensor("mask_neg", [128, 72], FP, kind="ExternalInput")
    out_t = nc.dram_tensor("out", [256, 36], FP, kind="ExternalOutput")

    with tile.TileContext(nc) as tc:
        with (tc.tile_pool(name="dram", bufs=1, space="DRAM") as dram,
              tc.tile_pool(name="const", bufs=1) as cp,
              tc.tile_pool(name="hin", bufs=3) as hp,
              tc.tile_pool(name="hps", bufs=2, space="PSUM") as hpp,
              tc.tile_pool(name="gath", bufs=1) as gp,
              tc.tile_pool(name="sel", bufs=3) as selp,
              tc.tile_pool(name="ps2", bufs=2, space="PSUM") as pp2,
              tc.tile_pool(name="fin", bufs=1) as fp_pool):
            # preload all index/select inputs (overlaps phase 1)
            idxs = cp.tile([128, Fp * 128 // 16], I16)
            nc.sync.dma_start(idxs[:], idx_in[:])
            offs = cp.tile([128, Fp], FP)
            nc.sync.dma_start(offs[:], off_in[:])
            iot = cp.tile([128, 16], FP)
            nc.sync.dma_start(iot[:], iota_in[:])
            m2 = cp.tile([128, 64], F16)
            nc.sync.dma_start(m2[:], m2_in[:])

            # ---------- phase 1: hv = h @ W (PE, fp16) ----------
            w0 = cp.tile([96, 128], F16)
            nc.sync.dma_start(w0[:], w0_in[:])
            w1 = cp.tile([96, 128], F16)
            nc.sync.dma_start(w1[:], w1_in[:])
            hv_part = dram.tile([NSHP], FP, tag="hvp")
            for ld in range(NL):
                lsl = slice(ld * HCHUNK * HLOAD, (ld + 1) * HCHUNK * HLOAD)
                h0 = hp.tile([96, HCHUNK * HLOAD], F16, tag="h0")
                nc.sync.dma_start(h0[:], ht0_in[:, lsl])
                h1 = hp.tile([96, HCHUNK * HLOAD], F16, tag="h1")
                nc.sync.dma_start(h1[:], ht1_in[:, lsl])
                for cc in range(HLOAD):
                    c = ld * HLOAD + cc
                    csl = slice(cc * HCHUNK, (cc + 1) * HCHUNK)
                    ps = hpp.tile([128, HCHUNK], FP, tag="ps1")
                    nc.tensor.matmul(out=ps[:], lhsT=w0[:], rhs=h0[:, csl],
                                     start=True, stop=False)
                    nc.tensor.matmul(out=ps[:], lhsT=w1[:], rhs=h1[:, csl],
                                     start=False, stop=True)
                    hst = hp.tile([1, HCHUNK], FP, tag="hst")
                    nc.vector.tensor_copy(out=hst[:], in_=ps[0:1, :])
                    nc.sync.dma_start(
                        hv_part[c * HCHUNK:(c + 1) * HCHUNK]
                        .rearrange("(o x) -> o x", o=1), hst[:])

            # ---------- AllGather ----------
            hv_full = dram.tile([NHV], FP, tag="hvf")
            nc.gpsimd.collective_compute(
                "AllGather", mybir.AluOpType.bypass,
                replica_groups=[list(range(NCORES))],
                ins=[hv_part.opt()], outs=[hv_full.opt()])

            # ---------- T4 table: each 16-block replicated 4x ----------
            hv_sb = cp.tile([128, NHV // 128], FP)       # [128, 784]
            nc.sync.dma_start(
                hv_sb[:], hv_full[:].rearrange("(p x) -> p x", p=128))
            t4 = cp.tile([128, (NHV // 128) * 4], FP)    # [128, 3136]
            nc.vector.tensor_copy(
                out=t4[:].rearrange("p (i r u) -> p i r u", r=4, u=16),
                in_=hv_sb[:].rearrange("p (i o u) -> p i o u", o=1, u=16)
                    .to_broadcast([128, NHV // 2048, 4, 16]))
            t4_dram = dram.tile([NROW, 64], FP, tag="t4")
            nc.sync.dma_start(
                t4_dram[:].rearrange("(p i) u -> p (i u)", p=128), t4[:])

            # ---------- gather + select + pair-reduce ----------
            iota3 = iot[:].rearrange("p (o c) -> p o c", o=1)
            ps2_d = dram.tile([64 * Fp], FP, tag="ps2d")
            ps2_v = ps2_d[:].rearrange("(p x) -> p x", p=64)
            for b in range(NBLK):
                gt = gp.tile([128, CB, 64], FP, tag="g")
                nc.gpsimd.dma_gather(
                    out_ap=gt[:], in_ap=t4_dram[:],
                    idxs_ap=idxs[:, b * (PERB // 16):(b + 1) * (PERB // 16)],
                    num_idxs=PERB, num_idxs_reg=PERB, elem_size=64,
                    single_packet=False, queue_num=1 + b % 3)
                oh = selp.tile([128, CB * 16], FP, tag="oh")
                nc.vector.tensor_tensor(
                    out=oh[:].rearrange("p (c o) -> p c o", o=16),
                    in0=offs[:, b * CB:(b + 1) * CB].to_broadcast(
                        [128, CB, 16]),
                    in1=iota3.to_broadcast([128, CB, 16]),
                    op=mybir.AluOpType.is_equal)
                prod = selp.tile([128, CB * 16], FP, tag="prod")
                nc.vector.tensor_tensor(
                    out=prod[:].rearrange("p (c o) -> p c o", o=16),
                    in0=gt[:, :, 0:16],
                    in1=oh[:].rearrange("p (c o) -> p c o", o=16),
                    op=mybir.AluOpType.mult)
                val = selp.tile([128, CB], F16, tag="val")
                with nc.allow_low_precision(reason="16-wide select, fp16 ok"):
                    nc.vector.tensor_reduce(
                        out=val[:],
                        in_=prod[:].rearrange("p (c o) -> p c o", o=16),
                        axis=mybir.AxisListType.X, op=mybir.AluOpType.add)
                pst = pp2.tile([64, CB], FP, tag="pst")
                nc.tensor.matmul(
                    out=pst[:], lhsT=m2[:], rhs=val[:],
                    start=True, stop=True)
                pss = selp.tile([64, CB], FP, tag="pss")
                nc.vector.tensor_copy(out=pss[:], in_=pst[:])
                nc.sync.dma_start(ps2_v[:, b * CB:(b + 1) * CB], pss[:])

            # ---------- repack + K-reduce + ext -> per-core bins ----------
            sb2 = fp_pool.tile([128, BPP * K], FP, tag="sb2")
            nc.sync.dma_start(sb2[:], ps2_v[:, 0:FpR])
            sbx = fp_pool.tile([128, EXTP], FP, tag="sbx")
            nc.sync.dma_start(sbx[:], ps2_v[:, FpR:FpR + EXTC])
            bins = fp_pool.tile([128, BPP], FP, tag="bins")
            nc.vector.tensor_reduce(
                out=bins[:],
                in_=sb2[:].rearrange("p (e k) -> p e k", k=K),
                axis=mybir.AxisListType.X, op=mybir.AluOpType.add)
            extm = cp.tile([128, EXTP * 72], FP)
            nc.sync.dma_start(extm[:], extm_in[:])
            for jx in range(EXTP):
                xt = fp_pool.tile([128, 72], FP, tag="xt")
                nc.vector.tensor_scalar(
                    out=xt[:], in0=extm[:, jx * 72:(jx + 1) * 72],
                    scalar1=sbx[:, jx:jx + 1], scalar2=None,
                    op0=mybir.AluOpType.mult)
                nc.vector.tensor_tensor(
                    out=bins[:], in0=bins[:], in1=xt[:],
                    op=mybir.AluOpType.add)

            # ---------- AllReduce + mask + softmax ----------
            part_d = dram.tile([NBINS], FP, tag="part")
            nc.sync.dma_start(
                part_d[:].rearrange("(p e) -> p e", p=128), bins[:])
            red_d = dram.tile([NBINS], FP, tag="red")
            nc.gpsimd.collective_compute(
                "AllReduce", mybir.AluOpType.add,
                replica_groups=[list(range(NCORES))],
                ins=[part_d.opt()], outs=[red_d.opt()])
            a_sb = fp_pool.tile([36, 256], FP, tag="asb")
            nc.sync.dma_start(
                a_sb[:], red_d[:].rearrange("(t g) -> t g", t=36))
            eye = cp.tile([36, 36], FP)
            nc.sync.dma_start(eye[:], eye_in[:])
            m0 = cp.tile([128, 72], FP)
            nc.sync.dma_start(m0[:], m0_in[:])
            mn = cp.tile([128, 72], FP)
            nc.sync.dma_start(mn[:], mn_in[:])
            for g in range(2):
                tp = hpp.tile([128, 36], FP, tag="tp")
                nc.tensor.transpose(
                    out=tp[:], in_=a_sb[:, g * 128:(g + 1) * 128],
                    identity=eye[:])
                gv = fp_pool.tile([128, 36], FP, tag="gv")
                nc.vector.tensor_tensor(
                    out=gv[:], in0=tp[:], in1=m0[:, g * 36:(g + 1) * 36],
                    op=mybir.AluOpType.mult)
                nc.vector.tensor_tensor(
                    out=gv[:], in0=gv[:], in1=mn[:, g * 36:(g + 1) * 36],
                    op=mybir.AluOpType.add)
                mx = fp_pool.tile([128, 1], FP, tag="mx")
                nc.vector.tensor_reduce(
                    out=mx[:], in_=gv[:],
                    axis=mybir.AxisListType.X, op=mybir.AluOpType.max)
                gvs = fp_pool.tile([128, 36], FP, tag="gvs")
                nc.vector.tensor_scalar(
                    out=gvs[:], in0=gv[:], scalar1=mx[:], scalar2=None,
                    op0=mybir.AluOpType.subtract)
                ex = fp_pool.tile([128, 36], FP, tag="ex")
                sm = fp_pool.tile([128, 1], FP, tag="sm")
                nc.scalar.activation(
                    out=ex[:], in_=gvs[:],
                    func=mybir.ActivationFunctionType.Exp,
                    accum_out=sm[:])
                rec = fp_pool.tile([128, 1], FP, tag="rec")
                nc.vector.reciprocal(rec[:], sm[:])
                res = fp_pool.tile([128, 36], FP, tag="res")
                nc.vector.tensor_scalar(
                    out=res[:], in0=ex[:], scalar1=rec[:], scalar2=None,
                    op0=mybir.AluOpType.mult)
                nc.sync.dma_start(out_t[g * 128:(g + 1) * 128, :], res[:])

    nc.compile()
    _split_multi_waits(nc)
    return nc


def kernel(h, W_out, edge_src, edge_seg, mask_mat):
    from concourse.bass_utils import run_bass_kernel_spmd

    h = np.ascontiguousarray(h, np.float32)
    W_out = np.ascontiguousarray(W_out, np.float32)
    K, per_core = _prepare_edges(edge_src, edge_seg)
    Fp = -(-(BPO * K + EXTC) // NBLK) * NBLK

    iota16 = np.broadcast_to(np.arange(16, dtype=np.float32), (128, 16)).copy()
    eye36 = np.eye(36, dtype=np.float32)
    mask2 = np.zeros((128, 64), np.float16)
    mask2[np.arange(128), np.arange(128) // 2] = 1.0

    def _mask_layout(m):
        return np.ascontiguousarray(
            m.reshape(2, 128, 36).transpose(1, 0, 2).reshape(128, 72))
    mask_keep = _mask_layout((~mask_mat).astype(np.float32))
    mask_neg = _mask_layout(mask_mat.astype(np.float32) * np.float32(-1e9))

    w16 = W_out[:, 0].astype(np.float16)
    w0 = np.broadcast_to(w16[0:96, None], (96, 128)).copy()
    w1 = np.broadcast_to(w16[96:192, None], (96, 128)).copy()

    in_maps = []
    for k in range(NCORES):
        hs = np.zeros((NSHP, D), np.float32)
        hs[:NSH] = h[k * NSH:(k + 1) * NSH]
        h16 = hs.astype(np.float16)
        in_maps.append({
            "ht0": np.ascontiguousarray(h16[:, 0:96].T),
            "ht1": np.ascontiguousarray(h16[:, 96:192].T),
            "w0": w0,
            "w1": w1,
            "idx16": _wrap_idx(per_core[k]["row"], Fp),
            "off": per_core[k]["off"],
            "iota16": iota16,
            "mask2": mask2,
            "extm": np.ascontiguousarray(
                per_core[k]["extm"].transpose(1, 0, 2).reshape(128, -1)),
            "eye36": eye36,
            "mask_keep": mask_keep,
            "mask_neg": mask_neg,
        })

    nc = _build_program(K)
    kwargs = {}
    if TRACE[0]:
        import tempfile
        kwargs = dict(trace=True, tmpdir=tempfile.mkdtemp(prefix="bondout_"))
    res = run_bass_kernel_spmd(nc, in_maps, core_ids=list(range(NCORES)),
                               **kwargs)
    LAST_EXEC_NS[0] = res.exec_time_ns
    return np.asarray(res.results[0]["out"], np.float32)


# revision 32
# speedup vs baseline: 5.1356x; 5.1356x over previous
"""Trainium2 Bass kernel for nn_BondOutputModule (gnn_message_passing).

Reference computation:
    hv = h @ W_out                       (projection pulled before segment sum)
    out[t,b] = sum_{e in type t, graph b} hv[src_e]
    graph_v[b,t]; mask; softmax over t

Device strategy (8 cores, SPMD), v9 "expand":
  The chip-shared DMA-descriptor rate (~150M desc/s/core at 8 cores) makes
  any per-edge dma_gather design bottom out near 1ms, so this version does
  ZERO gathers.  Edges live on the core owning their source node.
  - Per core, nodes are sorted by (expand) edge count; columns of 128 sorted
    nodes get a shared per-column slot count K_i (global max over cores, so
    the program geometry is SPMD-identical; fill ~93%).
  - Phase 1: h rows (host-permuted fp16) -> DVE mult + ACT accum-reduce ->
    hv32 [128, 98] (node (p,i) at sigma position 128i+p).  No AllGather.
  - Phase 2, per chunk (column i, slot k): one PE matmul accumulates
    lhsT = onehot36(type)*hv (built on DVE, batched) with
    rhs = onehot256(seg) (DVE, batched) into a persistent psum [36, 256]:
    bins[t, g] += hv[src] for every edge.  Pads use seg=300/typ=40 -> zero
    one-hot rows.  ~1150 matmuls total (~400ns each measured).
  - Tail: psum -> [36,256] -> AllReduce [9216] -> PE transpose -> mask ->
    softmax (same as before).
"""
import sys

if '/opt/trn_rl_repo' not in sys.path:
    sys.path.insert(0, '/opt/trn_rl_repo')

import numpy as np

TRACE = [False]          # test harness can set kernel.TRACE[0] = True
LAST_EXEC_NS = [None]    # filled when TRACE is on

N = 100000
D = 192
T = 36
E = 30000
B = 256
NCORES = 8
NSH = N // NCORES          # 12500 nodes per core
NSHP = 12544               # padded to 98*128
NCOL = NSHP // 128         # 98 node columns
CPB = 64                   # chunks per DVE one-hot batch
HLOAD = 8                  # h tiles per DMA
PAD_SEG = 300.0
PAD_TYP = 40.0


def _patch_tile_drain():
    """This walrus build accepts at most one sync-wait per CTRL/DMA
    instruction; Tile's tail drain can carry one wait per DMA lane."""
    import concourse.tile as tile
    from concourse.vector_clock import ScopedClock
    from concourse import mybir

    if getattr(tile.TileContext, '_bondout_patched', False):
        return

    def _drain_and_barrier(self, tick_clock, wait_clock):
        nc = self.nc
        carriers = [nc.sync.nop(nofuse=True, hint=f"dw{i}") for i in range(24)]
        drain_inst = nc.sync.drain()
        wait_clock.add_sem_waits(
            drain_inst.ins, ScopedClock({None: tick_clock.global_clock})
        )
        waits = list(drain_inst.ins.sync_info.on_wait)
        if len(waits) > 1:
            drain_inst.ins.sync_info.on_wait = waits[-1:]
            for c, w in zip(carriers, waits[:-1]):
                if c.ins.sync_info is None:
                    c.ins.sync_info = mybir.SyncInfo(on_wait=[w], on_update=[])
                else:
                    c.ins.sync_info.on_wait = [w]
        nc.all_engine_barrier()
        assert self.sems is not None
        popped = nc._tile_sem_poison_stack.pop()
        assert popped is self._sem_poison
        nc.clear_and_free_semaphores(list(self.sems.allocated().values()))
        nc.all_engine_barrier()

    tile.TileContext._drain_and_barrier = _drain_and_barrier
    tile.TileContext._bondout_patched = True


def _split_multi_waits(nc):
    from concourse import mybir
    for f in nc.m.functions:
        for blk in f.blocks:
            new = []
            changed = False
            for inst in blk.instructions:
                si = inst.sync_info
                if si is not None and si.on_wait and len(si.on_wait) > 1:
                    waits = list(si.on_wait)
                    for j, w in enumerate(waits[:-1]):
                        nop = mybir.InstNoOp(
                            name=f"{inst.name}-ws{j}",
                            engine=inst.engine,
                            bass_nofuse=True,
                            sync_info=mybir.SyncInfo(on_wait=[w], on_update=[]),
                        )
                        new.append(nop)
                    si.on_wait = waits[-1:]
                    changed = True
                new.append(inst)
            if changed:
                blk.instructions = new


def _prepare_edges(edge_src, edge_seg):
    """Per-core node sort by edge count + shared column-K profile + per-slot
    seg/type arrays.

    Returns (Kcols, per_core) with Kcols[i] = slots for node-column i and
    per_core[k] = dict(perm = sigma (node order for h upload),
                       seg = [128, NCH] f32, typ = [128, NCH] f32).
    """
    src = edge_src.astype(np.int64).reshape(-1)
    typ = np.repeat(np.arange(T, dtype=np.int64), E)
    seg = edge_seg.astype(np.int64).reshape(-1)
    core = src // NSH
    nl = src - core * NSH

    per_core_raw = []
    cnt_sorted = np.zeros((NCORES, NSHP), np.int64)
    for k in range(NCORES):
        m = core == k
        cnt = np.bincount(nl[m], minlength=NSHP)
        sigma = np.argsort(-cnt, kind="stable")          # node order
        cnt_sorted[k] = cnt[sigma]
        per_core_raw.append((m, sigma))

    # shared K profile: per column of 128 sorted nodes, max count over cores
    Kcols = cnt_sorted.reshape(NCORES, NCOL, 128).max(axis=2).max(axis=0)
    Kcols = np.maximum(Kcols, 0)
    NCH = int(Kcols.sum())
    col_start = np.concatenate([[0], np.cumsum(Kcols)])

    per_core = []
    for k in range(NCORES):
        m, sigma = per_core_raw[k]
        spos = np.empty(NSHP, np.int64)
        spos[sigma] = np.arange(NSHP)                    # node -> sigma pos
        sp = spos[nl[m]]                                 # [edges] sigma pos
        p = sp % 128
        i = sp // 128
        # rank of each edge within its node
        order = np.argsort(sp, kind="stable")
        sps = sp[order]
        starts = np.zeros(NSHP, np.int64)
        starts[1:] = np.cumsum(np.bincount(sps, minlength=NSHP))[:-1]
        rank_sorted = np.arange(len(sps)) - starts[sps]
        ksl = np.empty(len(sps), np.int64)
        ksl[order] = rank_sorted
        ch = col_start[i] + ksl
        seg_a = np.full((128, NCH), PAD_SEG, np.float32)
        typ_a = np.full((128, NCH), PAD_TYP, np.float32)
        seg_a[p, ch] = seg[m]
        typ_a[p, ch] = typ[m]
        per_core.append({"perm": sigma, "seg": seg_a, "typ": typ_a})
    return Kcols, NCH, per_core


def _build_program(Kcols, NCH):
    import concourse.bass as bass
    from concourse import bacc, mybir
    import concourse.tile as tile

    _patch_tile_drain()
    FP = mybir.dt.float32
    F16 = mybir.dt.float16
    NB = -(-NCH // CPB)
    col_start = np.concatenate([[0], np.cumsum(Kcols)]).astype(int)

    nc = bacc.Bacc(num_swdge_queues=4)
    h_in = nc.dram_tensor("hh", [128, NCOL * D], F16, kind="ExternalInput")
    w_in = nc.dram_tensor("wb", [128, D], F16, kind="ExternalInput")
    seg_in = nc.dram_tensor("segs", [128, NCH], F16, kind="ExternalInput")
    typ_in = nc.dram_tensor("typs", [128, NCH], F16, kind="ExternalInput")
    i256_in = nc.dram_tensor("iota256", [128, 256], F16, kind="ExternalInput")
    i36_in = nc.dram_tensor("iota36", [128, 36], F16, kind="ExternalInput")
    eye_in = nc.dram_tensor("eye36", [36, 36], FP, kind="ExternalInput")
    m0_in = nc.dram_tensor("mask_keep", [128, 72], FP, kind="ExternalInput")
    mn_in = nc.dram_tensor("mask_neg", [128, 72], FP, kind="ExternalInput")
    out_t = nc.dram_tensor("out", [256, 36], FP, kind="ExternalOutput")

    with tile.TileContext(nc) as tc:
        with (tc.tile_pool(name="dram", bufs=1, space="DRAM") as dram,
              tc.tile_pool(name="const", bufs=1) as cp,
              tc.tile_pool(name="hin", bufs=3) as hp,
              tc.tile_pool(name="oh", bufs=3) as ohp,
              tc.tile_pool(name="ps", bufs=1, space="PSUM") as pp,
              tc.tile_pool(name="tp", bufs=1, space="PSUM") as tpp,
              tc.tile_pool(name="fin", bufs=1) as fp_pool):
            # preload constants / slot metadata
            segs = cp.tile([128, NCH], F16)
            nc.sync.dma_start(segs[:], seg_in[:])
            typs = cp.tile([128, NCH], F16)
            nc.sync.dma_start(typs[:], typ_in[:])
            i256 = cp.tile([128, 256], F16)
            nc.sync.dma_start(i256[:], i256_in[:])
            i36 = cp.tile([128, 36], F16)
            nc.sync.dma_start(i36[:], i36_in[:])
            wb = cp.tile([128, D], F16)
            nc.sync.dma_start(wb[:], w_in[:])
            i3 = i256[:].rearrange("p (o c) -> p o c", o=1)
            i363 = i36[:].rearrange("p (o c) -> p o c", o=1)

            # ---------- phase 1: hv32[p, i] = h[128i+p] . w ----------
            hv32 = cp.tile([128, NCOL], FP)
            for ld in range(NCOL // HLOAD + (NCOL % HLOAD > 0)):
                i0 = ld * HLOAD
                nt = min(HLOAD, NCOL - i0)
                ht = hp.tile([128, HLOAD * D], F16, tag="ht")
                nc.sync.dma_start(
                    ht[:, 0:nt * D], h_in[:, i0 * D:(i0 + nt) * D])
                for ii in range(nt):
                    i = i0 + ii
                    scr = hp.tile([128, D], F16, tag="scr")
                    nc.vector.tensor_tensor(
                        out=scr[:], in0=ht[:, ii * D:(ii + 1) * D],
                        in1=wb[:], op=mybir.AluOpType.mult)
                    dump = hp.tile([128, D], F16, tag="dump")
                    nc.scalar.activation(
                        out=dump[:], in_=scr[:],
                        func=mybir.ActivationFunctionType.Copy,
                        accum_out=hv32[:, i:i + 1])

            # ---------- phase 2: chunked one-hot matmul scatter ----------
            ps = pp.tile([36, 256], FP)
            # batched one-hot builds + per-column lhsT val-mult
            for bb in range(NB):
                c0, c1 = bb * CPB, min((bb + 1) * CPB, NCH)
                nn = c1 - c0
                oh = ohp.tile([128, CPB * 256], F16, tag="oh")
                nc.vector.tensor_tensor(
                    out=oh[:, 0:nn * 256].rearrange("p (c o) -> p c o", o=256),
                    in0=segs[:, c0:c1].to_broadcast([128, nn, 256]),
                    in1=i3.to_broadcast([128, nn, 256]),
                    op=mybir.AluOpType.is_equal)
                lh = ohp.tile([128, CPB * 36], F16, tag="lh")
                nc.vector.tensor_tensor(
                    out=lh[:, 0:nn * 36].rearrange("p (c o) -> p c o", o=36),
                    in0=typs[:, c0:c1].to_broadcast([128, nn, 36]),
                    in1=i363.to_broadcast([128, nn, 36]),
                    op=mybir.AluOpType.is_equal)
                # multiply by hv per column (columns covering [c0, c1))
                lv = ohp.tile([128, CPB * 36], F16, tag="lv")
                ic0 = int(np.searchsorted(col_start, c0, side="right")) - 1
                ic1 = int(np.searchsorted(col_start, c1, side="left"))
                for i in range(ic0, ic1):
                    a = max(c0, int(col_start[i]))
                    b_ = min(c1, int(col_start[i + 1]))
                    if a >= b_:
                        continue
                    with nc.allow_low_precision(reason="fp16 edge vals"):
                        nc.vector.tensor_tensor(
                            out=lv[:, (a - c0) * 36:(b_ - c0) * 36]
                                .rearrange("p (c o) -> p c o", o=36),
                            in0=lh[:, (a - c0) * 36:(b_ - c0) * 36]
                                .rearrange("p (c o) -> p c o", o=36),
                            in1=hv32[:, i:i + 1]
                                .rearrange("p (c o) -> p c o", o=1)
                                .to_broadcast([128, b_ - a, 36]),
                            op=mybir.AluOpType.mult)
                for c in range(c0, c1):
                    nc.tensor.matmul(
                        out=ps[0:36, :],
                        lhsT=lv[:, (c - c0) * 36:(c - c0 + 1) * 36],
                        rhs=oh[:, (c - c0) * 256:(c - c0 + 1) * 256],
                        start=(c == 0), stop=(c == NCH - 1),
                        skip_group_check=True)

            # ---------- AllReduce + mask + softmax ----------
            sb = fp_pool.tile([36, 256], FP, tag="sb")
            nc.vector.tensor_copy(out=sb[:], in_=ps[:])
            part_d = dram.tile([T * B], FP, tag="part")
            nc.sync.dma_start(
                part_d[:].rearrange("(t g) -> t g", t=36), sb[:])
            red_d = dram.tile([T * B], FP, tag="red")
            nc.gpsimd.collective_compute(
                "AllReduce", mybir.AluOpType.add,
                replica_groups=[list(range(NCORES))],
                ins=[part_d.opt()], outs=[red_d.opt()])
            a_sb = fp_pool.tile([36, 256], FP, tag="asb")
            nc.sync.dma_start(
                a_sb[:], red_d[:].rearrange("(t g) -> t g", t=36))
            eye = cp.tile([36, 36], FP)
            nc.sync.dma_start(eye[:], eye_in[:])
            m0 = cp.tile([128, 72], FP)
            nc.sync.dma_start(m0[:], m0_in[:])
            mn = cp.tile([128, 72], FP)
            nc.sync.dma_start(mn[:], mn_in[:])
            for g in range(2):
                tp = tpp.tile([128, 36], FP, tag="tp")
                nc.tensor.transpose(
                    out=tp[:], in_=a_sb[:, g * 128:(g + 1) * 128],
                    identity=eye[:])
                gv = fp_pool.tile([128, 36], FP, tag="gv")
                nc.vector.tensor_tensor(
                    out=gv[:], in0=tp[:], in1=m0[:, g * 36:(g + 1) * 36],
                    op=mybir.AluOpType.mult)
                nc.vector.tensor_tensor(
                    out=gv[:], in0=gv[:], in1=mn[:, g * 36:(g + 1) * 36],
                    op=mybir.AluOpType.add)
                mx = fp_pool.tile([128, 1], FP, tag="mx")
                nc.vector.tensor_reduce(
                    out=mx[:], in_=gv[:],
                    axis=mybir.AxisListType.X, op=mybir.AluOpType.max)
                gvs = fp_pool.tile([128, 36], FP, tag="gvs")
                nc.vector.tensor_scalar(
                    out=gvs[:], in0=gv[:], scalar1=mx[:], scalar2=None,
                    op0=mybir.AluOpType.subtract)
                ex = fp_pool.tile([128, 36], FP, tag="ex")
                sm = fp_pool.tile([128, 1], FP, tag="sm")
                nc.scalar.activation(
                    out=ex[:], in_=gvs[:],
                    func=mybir.ActivationFunctionType.Exp,
                    accum_out=sm[:])
                rec = fp_pool.tile([128, 1], FP, tag="rec")
                nc.vector.reciprocal(rec[:], sm[:])
                res = fp_pool.tile([128, 36], FP, tag="res")
                nc.vector.tensor_scalar(
                    out=res[:], in0=ex[:], scalar1=rec[:], scalar2=None,
                    op0=mybir.AluOpType.mult)
                nc.sync.dma_start(out_t[g * 128:(g + 1) * 128, :], res[:])

    nc.compile()
    _split_multi_waits(nc)
    return nc


def kernel(h, W_out, edge_src, edge_seg, mask_mat):
    from concourse.bass_utils import run_bass_kernel_spmd

    h = np.ascontiguousarray(h, np.float32)
    W_out = np.ascontiguousarray(W_out, np.float32)
    Kcols, NCH, per_core = _prepare_edges(edge_src, edge_seg)

    i256 = np.broadcast_to(np.arange(256, dtype=np.float16), (128, 256)).copy()
    i36 = np.broadcast_to(np.arange(36, dtype=np.float16), (128, 36)).copy()
    eye36 = np.eye(36, dtype=np.float32)
    wb = np.broadcast_to(W_out[:, 0].astype(np.float16), (128, D)).copy()

    def _mask_layout(m):
        return np.ascontiguousarray(
            m.reshape(2, 128, 36).transpose(1, 0, 2).reshape(128, 72))
    mask_keep = _mask_layout((~mask_mat).astype(np.float32))
    mask_neg = _mask_layout(mask_mat.astype(np.float32) * np.float32(-1e9))

    in_maps = []
    for k in range(NCORES):
        hs = np.zeros((NSHP, D), np.float32)
        hs[:NSH] = h[k * NSH:(k + 1) * NSH]
        hperm = hs[per_core[k]["perm"]].astype(np.float16)
        hh = np.ascontiguousarray(
            hperm.reshape(NCOL, 128, D).transpose(1, 0, 2).reshape(128, -1))
        in_maps.append({
            "hh": hh,
            "wb": wb,
            "segs": per_core[k]["seg"].astype(np.float16),
            "typs": per_core[k]["typ"].astype(np.float16),
            "iota256": i256,
            "iota36": i36,
            "eye36": eye36,
            "mask_keep": mask_keep,
            "mask_neg": mask_neg,
        })

    nc = _build_program(Kcols, NCH)
    kwargs = {}
    if TRACE[0]:
        import tempfile
        kwargs = dict(trace=True, tmpdir=tempfile.mkdtemp(prefix="bondout_"))
    res = run_bass_kernel_spmd(nc, in_maps, core_ids=list(range(NCORES)),
                               **kwargs)
    LAST_EXEC_NS[0] = res.exec_time_ns
    return np.asarray(res.results[0]["out"], np.float32)
